# revision 16
# baseline (speedup 1.0000x reference)
"""DeepFilter kernel for Trainium2 (8 NeuronCores, batch-parallel).

Math: the reference shifts input and filter by the SAME (df, dt) tap offset,
so the op factorizes into pointwise products followed by a separable 3x5
zero-padded box sum:
    P_r = ir*fr - ii*fi ; P_i = 2*ir*fi
    out_r = boxsum_3x5(P_r) ; out_i = boxsum_3x5(P_i)
    out = concat([out_r, out_i], axis=1)            # [B, 2F, T]

Per-core layout: F on partitions (3 chunks), T on the free dim (2 halves of
2000 + 2-col halo).  DVE computes the 3 product planes; TensorE applies the
F-box (banded 128x128 matmul, sign/scale folded into the band) x 5 shifted
rhs reads (T-box) accumulating in PSUM; ScalarE copies PSUM->SBUF; HWDGE
DMAs stream HBM.
"""

import numpy as np

B, F, T = 16, 257, 4000
NCORES = 8
B_LOC = B // NCORES  # 2
P = 128
TH = 2000  # T half processed per macro-tile
TW = TH + 4  # with 2-col halo each side
NJ = 4  # psum tiles per half
NT = 500  # psum tile width (<=512 fp32 matmul moving-operand limit)

# F chunks: (first loaded row, n rows loaded, valid psum partitions [lo,hi),
#            first output f row)
#  c0: rows 0..127   -> f 0..126  at partitions 0..126
#  c1: rows 126..253 -> f 127..252 at partitions 1..126
#  c2: rows 251..256 -> f 253..256 at partitions 0..3 (K=6 matmul, lhsT slice)
CHUNKS = [
    (0, 128, 0, 127, 0),
    (126, 128, 1, 127, 127),
    (251, 6, 0, 4, 253),
]

_CACHE = {}


def _band_matrices():
    k = np.arange(P)
    band = (np.abs(k[:, None] - k[None, :]) <= 1).astype(np.float32)
    return band, -band


def _build_module(repeats=1):
    import concourse.bacc as bacc
    import concourse.mybir as mybir
    import concourse.tile as tile

    f32 = mybir.dt.float32
    f32r = mybir.dt.float32r
    mult = mybir.AluOpType.mult

    nc = bacc.Bacc("TRN2", target_bir_lowering=False, debug=False,
                   num_devices=NCORES)

    ins = {
        name: nc.dram_tensor(name, [B_LOC, F, T], f32, kind="ExternalInput")
        for name in ("inputs_r", "inputs_i", "filters_r", "filters_i")
    }
    wp_d = nc.dram_tensor("wp", [P, P], f32r, kind="ExternalInput")
    wm_d = nc.dram_tensor("wm", [P, P], f32r, kind="ExternalInput")
    out_d = nc.dram_tensor("out", [B_LOC, 2 * F, T], f32, kind="ExternalOutput")

    ir_ap, ii_ap, fr_ap, fi_ap = (ins[n].ap() for n in
                                  ("inputs_r", "inputs_i", "filters_r",
                                   "filters_i"))
    out_ap = out_d.ap()

    with tile.TileContext(nc) as tc:
        with (
            tc.tile_pool(name="const", bufs=1) as cpool,
            tc.tile_pool(name="inp", bufs=8) as ipool,
            tc.tile_pool(name="prod", bufs=6) as rpool,
            tc.tile_pool(name="stg", bufs=4) as spool,
            tc.tile_pool(name="ps", bufs=8, space="PSUM") as qpool,
        ):
            wp_s = cpool.tile([P, P], f32r, name="wp_s", tag="wp_s")
            wm_s = cpool.tile([P, P], f32r, name="wm_s", tag="wm_s")
            nc.sync.dma_start(out=wp_s[:, :], in_=wp_d.ap()[:, :])
            nc.sync.dma_start(out=wm_s[:, :], in_=wm_d.ap()[:, :])

            for _rep in range(repeats):
              for b in range(B_LOC):
                for ci, (fl0, nrows, vp0, vp1, fo0) in enumerate(CHUNKS):
                    for h in range(2):
                        t0 = TH * h
                        # tile col c <-> t = t0 - 2 + c ; clip to [0, T)
                        c_lo = 2 if h == 0 else 0
                        c_hi = TW if t0 + TH + 2 <= T else TH + 2
                        t_lo, t_hi = t0 - 2 + c_lo, t0 - 2 + c_hi

                        ir_t = ipool.tile([P, TW], f32, name="ir_t", tag="inp")
                        ii_t = ipool.tile([P, TW], f32, name="ii_t", tag="inp")
                        fr_t = ipool.tile([P, TW], f32, name="fr_t", tag="inp")
                        fi_t = ipool.tile([P, TW], f32, name="fi_t", tag="inp")
                        for t_sb, src in ((ir_t, ir_ap), (ii_t, ii_ap),
                                          (fr_t, fr_ap), (fi_t, fi_ap)):
                            nc.sync.dma_start(
                                out=t_sb[0:nrows, c_lo:c_hi],
                                in_=src[b, fl0:fl0 + nrows, t_lo:t_hi])
                            # zero halo cols at the global T edges so the
                            # products are zero there (zero-pad semantics)
                            # and matmuls can always run full-width
                            if c_lo > 0:
                                nc.vector.memset(t_sb[0:nrows, 0:c_lo], 0.0)
                            if c_hi < TW:
                                nc.vector.memset(t_sb[0:nrows, c_hi:TW], 0.0)

                        # float32r: PE matmuls on fp32r run 4x faster than
                        # fp32; DVE rounds the products on write.
                        t1_t = rpool.tile([P, TW], f32r, name="t1_t", tag="prod")
                        t2_t = rpool.tile([P, TW], f32r, name="t2_t", tag="prod")
                        pi_t = rpool.tile([P, TW], f32r, name="pi_t", tag="prod")
                        nc.vector.tensor_mul(t1_t[0:nrows, 0:TW],
                                             ir_t[0:nrows, 0:TW],
                                             fr_t[0:nrows, 0:TW])
                        nc.vector.tensor_mul(t2_t[0:nrows, 0:TW],
                                             ii_t[0:nrows, 0:TW],
                                             fi_t[0:nrows, 0:TW])
                        # pi = (ir * 2) * fi -- fold the reference's factor 2
                        nc.vector.scalar_tensor_tensor(
                            out=pi_t[0:nrows, 0:TW],
                            in0=ir_t[0:nrows, 0:TW], scalar=2.0,
                            in1=fi_t[0:nrows, 0:TW], op0=mult, op1=mult)

                        if ci < 2:
                            wpL, wmL, np_out = wp_s[:, :], wm_s[:, :], P
                        else:
                            # banded [6,4] slice: W6[k,m] = band[k, m+2]
                            wpL = wp_s[0:6, 2:6]
                            wmL = wm_s[0:6, 2:6]
                            np_out = 4

                        stg_r = spool.tile([P, TH], f32, name="stg_r", tag="stg")
                        stg_i = spool.tile([P, TH], f32, name="stg_i", tag="stg")

                        for j in range(NJ):
                            ps_r = qpool.tile([P, NT], f32, name="ps_r", tag="ps")
                            ps_i = qpool.tile([P, NT], f32, name="ps_i", tag="ps")
                            # weight order wp (pi), wp (t1), wm (t2):
                            # only 2 LDWEIGHTS switches per j
                            for ps, planes in (
                                (ps_i, ((pi_t, wpL),)),
                                (ps_r, ((t1_t, wpL), (t2_t, wmL))),
                            ):
                                mms = []
                                for plane, wL in planes:
                                    for dj in (-2, -1, 0, 1, 2):
                                        c_start = NT * j + 2 + dj
                                        mms.append((plane, wL, c_start))
                                for k, (plane, wL, c_start) in enumerate(mms):
                                    nc.tensor.matmul(
                                        ps[0:np_out, 0:NT],
                                        wL,
                                        plane[0:nrows, c_start:c_start + NT],
                                        start=(k == 0),
                                        stop=(k == len(mms) - 1))
                            # PSUM reads must start at partition 0: copy
                            # rows 0:vp1 and let the store DMA pick vp0:vp1.
                            nc.scalar.copy(
                                out=stg_r[0:vp1, NT * j:NT * (j + 1)],
                                in_=ps_r[0:vp1, 0:NT])
                            nc.scalar.copy(
                                out=stg_i[0:vp1, NT * j:NT * (j + 1)],
                                in_=ps_i[0:vp1, 0:NT])

                        n_f = (vp1 - vp0)
                        nc.sync.dma_start(
                            out=out_ap[b, fo0:fo0 + n_f, t0:t0 + TH],
                            in_=stg_r[vp0:vp1, 0:TH])
                        nc.sync.dma_start(
                            out=out_ap[b, F + fo0:F + fo0 + n_f, t0:t0 + TH],
                            in_=stg_i[vp0:vp1, 0:TH])

    nc.compile()
    return nc


def _get_module(repeats=1):
    key = f"nc{repeats}"
    if key not in _CACHE:
        _CACHE[key] = _build_module(repeats)
    return _CACHE[key]


def kernel(trace=False, **inputs):
    from concourse.bass_utils import run_bass_kernel_spmd

    nc = _get_module()
    wp, wm = _band_matrices()
    in_maps = []
    for c in range(NCORES):
        sl = slice(c * B_LOC, (c + 1) * B_LOC)
        in_maps.append({
            "inputs_r": np.ascontiguousarray(inputs["inputs_r"][sl]),
            "inputs_i": np.ascontiguousarray(inputs["inputs_i"][sl]),
            "filters_r": np.ascontiguousarray(inputs["filters_r"][sl]),
            "filters_i": np.ascontiguousarray(inputs["filters_i"][sl]),
            "wp": wp,
            "wm": wm,
        })
    res = run_bass_kernel_spmd(nc, in_maps, core_ids=list(range(NCORES)),
                               trace=trace)
    _CACHE["last_result"] = res
    return np.concatenate([r["out"] for r in res.results], axis=0)
